# revision 1
# baseline (speedup 1.0000x reference)
"""Trainium2 Bass kernel for nn_CVRP_Encoder (AFT-style CVRP encoder).

Data-parallel over batch B=32 across 8 NeuronCores (4 items/core). Per item
everything lives in a transposed [D=128 (partitions), S=1000 (free)] layout so
instance-norm reduces along the free axis. S splits into 8 chunks of 125 for
the attention contraction (t on partitions) and 2 chunks of 500 for matmul
free dims.

Fusions:
 - norm-apply + residual-add in one DVE op (affine_then_add), so the fp32
   residual stream is carried as the pre-norm tensor y2 plus per-channel
   (A, C) affine coefficients.
 - sig*weighted multiply carries the free-axis sum (affine_mul_reduce), and
   mean(x) is known analytically from the previous norm's bias, so norm-1
   needs no explicit reduction pass.
 - bW2 is dropped: a per-channel constant shift is exactly cancelled by the
   following instance norm.
"""
import sys

sys.path.insert(0, "/opt/trn_rl_repo")

import numpy as np

import concourse.bass as bass
import concourse.tile as tile
from concourse import bacc, mybir
from concourse.bass_utils import run_bass_kernel_spmd

F32 = mybir.dt.float32
F16 = mybir.dt.float16
BF16 = mybir.dt.bfloat16
I32 = mybir.dt.int32
AF = mybir.ActivationFunctionType
ALU = mybir.AluOpType

B, N, D, F, L = 32, 999, 128, 512, 6
S = N + 1
P = 128
NCORES = 8
IPC = B // NCORES
TC = 8
TCS = S // TC      # 125
SC = 2
SCS = S // SC      # 500
FC = F // P        # 4
EPS = 1e-5
RSQRT_MAGIC = 0x5F3759DF + 1
GRP = 2            # items per norm-batching group


def _bcast_dram(handle, n_part, idx, count):
    ap = handle[:]
    return bass.AP(tensor=ap.tensor, offset=idx, ap=[[0, n_part], [1, count]])


def _nv(t):
    """[P, 1024] tile/psum -> [P, 2, 500] strided view (skip 512-pad)."""
    return t[:].rearrange("p (n s) -> p n s", n=2)[:, :, 0:SCS]


def _v2(t):
    """[P, S] tile -> [P, 2, 500] view."""
    return t[:].rearrange("p (n s) -> p n s", n=2)


def build_cvrp(cs):
    """cs: per-layer scale constants c_l = log_scale * alpha[l]."""
    shared_es = all(abs(c - cs[0]) < 1e-30 for c in cs)

    nc = bacc.Bacc("TRN2", target_bir_lowering=False, debug=False,
                   num_devices=NCORES)

    g = {}
    g["dist_t"] = nc.declare_dram_parameter("dist_t", [IPC, TC, TCS, S], BF16, isOutput=False)
    g["node_t"] = nc.declare_dram_parameter("node_t", [IPC, 3, N], F16, isOutput=False)
    g["depot"] = nc.declare_dram_parameter("depot", [IPC, 2], F32, isOutput=False)
    g["flagf"] = nc.declare_dram_parameter("flagf", [IPC], F32, isOutput=False)
    g["wqt"] = nc.declare_dram_parameter("wqt", [L, D, D], F16, isOutput=False)
    g["wkt"] = nc.declare_dram_parameter("wkt", [L, D, D], F16, isOutput=False)
    g["wvt"] = nc.declare_dram_parameter("wvt", [L, D, D], F16, isOutput=False)
    g["w1t"] = nc.declare_dram_parameter("w1t", [L, D, F], F16, isOutput=False)
    g["w2t"] = nc.declare_dram_parameter("w2t", [L, P, FC, D], F16, isOutput=False)
    g["wnt"] = nc.declare_dram_parameter("wnt", [3, D], F16, isOutput=False)
    g["wdt"] = nc.declare_dram_parameter("wdt", [2, D], F32, isOutput=False)
    g["wint"] = nc.declare_dram_parameter("wint", [D, D], F32, isOutput=False)
    g["woutt"] = nc.declare_dram_parameter("woutt", [D, D], F32, isOutput=False)
    g["biases4"] = nc.declare_dram_parameter("biases4", [D, 4], F32, isOutput=False)
    g["bw1_t"] = nc.declare_dram_parameter("bw1_t", [D, L, FC], F32, isOutput=False)
    g["g1_t"] = nc.declare_dram_parameter("g1_t", [D, L], F32, isOutput=False)
    g["b1_t"] = nc.declare_dram_parameter("b1_t", [D, L], F32, isOutput=False)
    g["g2_t"] = nc.declare_dram_parameter("g2_t", [D, L], F32, isOutput=False)
    g["b2_t"] = nc.declare_dram_parameter("b2_t", [D, L], F32, isOutput=False)
    g["out32"] = nc.declare_dram_parameter("out32", [IPC, D, S], F32, isOutput=True)

    with tile.TileContext(nc) as tc_ctx:
        _body(nc, tc_ctx, g, cs, shared_es)
    nc.compile()
    return nc


def _norm_smalls(nc, np_, sums, sumsq, g_col, b_col, tag, mean_bias=None,
                 mean_bias_cols=None):
    """Instance-norm scalar math on [D, GRP] tiles.
    mean = sums/S + mean_bias; var = sumsq/S + eps - mean^2; rstd via
    bit-trick + 2 Newton iters. Returns (A, C): out = A*y + C."""
    sm = np_.tile([D, 8, GRP], F32, tag=f"nsm_{tag}")
    mean, msq, var = sm[:, 0], sm[:, 1], sm[:, 2]
    if mean_bias is not None:
        nc.vector.tensor_scalar(mean, sums, 1.0 / S, mean_bias, ALU.mult, ALU.add)
    else:
        nc.vector.tensor_scalar(mean, sums, 1.0 / S, None, ALU.mult)
    if mean_bias_cols is not None:
        nc.vector.tensor_tensor(mean, mean, mean_bias_cols, ALU.add)
    nc.vector.tensor_tensor(msq, mean, mean, ALU.mult)
    nc.vector.tensor_scalar(var, sumsq, 1.0 / S, EPS, ALU.mult, ALU.add)
    nc.vector.tensor_tensor(var, var, msq, ALU.subtract)
    ry = sm[:, 3]
    ibits = ry.bitcast(I32)
    nc.vector.tensor_scalar(ibits, var.bitcast(I32), 1, -1,
                            ALU.logical_shift_right, ALU.bitwise_xor)
    nc.vector.tensor_scalar(ibits, ibits, RSQRT_MAGIC, None, ALU.add)
    t1, t2 = sm[:, 4], sm[:, 5]
    for _ in range(2):
        nc.vector.tensor_tensor(t1, ry, ry, ALU.mult)
        nc.vector.tensor_tensor(t2, t1, var, ALU.mult)
        nc.vector.tensor_scalar(t2, t2, -0.5, 1.5, ALU.mult, ALU.add)
        nc.vector.tensor_tensor(ry, ry, t2, ALU.mult)
    A, C = sm[:, 6], sm[:, 7]
    nc.vector.tensor_scalar(A, ry, g_col, None, ALU.mult)
    nc.vector.tensor_tensor(C, mean, A, ALU.mult)
    nc.vector.tensor_scalar(C, C, b_col, -1.0, ALU.subtract, ALU.mult)
    return A, C


def _body(nc, tc, g, cs, shared_es):
    from contextlib import ExitStack

    ctx = ExitStack()
    singles = ctx.enter_context(tc.tile_pool(name="singles", bufs=1))
    xpool = ctx.enter_context(tc.tile_pool(name="xpool", bufs=1))
    tp = ctx.enter_context(tc.tile_pool(name="tp", bufs=2))
    scr = ctx.enter_context(tc.tile_pool(name="scr", bufs=2))
    np_ = ctx.enter_context(tc.tile_pool(name="npool", bufs=2))
    pp = ctx.enter_context(tc.tile_pool(name="pp", bufs=1))
    ps = ctx.enter_context(tc.tile_pool(name="ps", bufs=4, space="PSUM"))

    # ---- resident weights ----
    t_wqt, t_wkt, t_wvt, t_w1t, t_w2t = [], [], [], [], []
    for l in range(L):
        for lst, src, shape in ((t_wqt, g["wqt"], [D, D]), (t_wkt, g["wkt"], [D, D]),
                                (t_wvt, g["wvt"], [D, D]), (t_w1t, g["w1t"], [D, F]),
                                (t_w2t, g["w2t"], [P, FC, D])):
            w = singles.tile(shape, F16, tag=f"w{id(lst)}_{l}")
            nc.sync.dma_start(w[:], src[l])
            lst.append(w)
    t_wnt = singles.tile([3, D], F16, tag="wnt")
    nc.sync.dma_start(t_wnt[:], g["wnt"][:])
    t_wdt = singles.tile([2, D], F32, tag="wdt")
    nc.sync.dma_start(t_wdt[:], g["wdt"][:])
    t_wint = singles.tile([D, D], F32, tag="wint")
    nc.sync.dma_start(t_wint[:], g["wint"][:])
    t_woutt = singles.tile([D, D], F32, tag="woutt")
    nc.sync.dma_start(t_woutt[:], g["woutt"][:])
    sm_t = {}
    for nm, shp in (("biases4", [D, 4]), ("bw1_t", [D, L, FC]), ("g1_t", [D, L]),
                    ("b1_t", [D, L]), ("g2_t", [D, L]), ("b2_t", [D, L])):
        t = singles.tile(shp, F32, tag=nm)
        nc.sync.dma_start(t[:], g[nm][:])
        sm_t[nm] = t
    t_b4, t_bw1 = sm_t["biases4"], sm_t["bw1_t"]
    t_g1, t_b1, t_g2, t_b2 = sm_t["g1_t"], sm_t["b1_t"], sm_t["g2_t"], sm_t["b2_t"]
    t_ff = singles.tile([P, IPC], F32, tag="ffl")
    nc.sync.dma_start(t_ff[:], _bcast_dram(g["flagf"], P, 0, IPC))
    emb_mean = singles.tile([D, IPC], F32, tag="embm")

    BD, BN_, BIN, BOUT = (t_b4[:, i : i + 1] for i in range(4))

    # ---- embedding ----
    x32s, x16s = [], []
    for i in range(IPC):
        x32 = xpool.tile([D, S], F32, tag=f"x32_{i}")
        t_node = scr.tile([P, 1024], F16, tag="node16")
        nc.sync.dma_start(t_node[:3, 0:N], g["node_t"][i])
        t_dep = tp.tile([2, 1], F32, tag="dep")
        nc.sync.dma_start(t_dep[:], g["depot"][i, :, None])
        pe = ps.tile([P, 1024], F32, tag="ps")
        nc.tensor.matmul(pe[:, 0:500], t_wnt[:], t_node[:3, 0:500], start=True, stop=True)
        nc.tensor.matmul(pe[:, 512:1011], t_wnt[:], t_node[:3, 500:999], start=True, stop=True)
        nc.scalar.activation(x32[:, 1:501], pe[:, 0:500], AF.Identity, bias=BN_, scale=1.0)
        nc.scalar.activation(x32[:, 501:1000], pe[:, 512:1011], AF.Identity, bias=BN_, scale=1.0)
        pd = ps.tile([P, 1024], F32, tag="ps")
        nc.tensor.matmul(pd[:, 0:1], t_wdt[:], t_dep[:], start=True, stop=True)
        nc.scalar.activation(x32[:, 0:1], pd[:, 0:1], AF.Identity, bias=BD, scale=1.0)
        pw = ps.tile([P, 1024], F32, tag="ps")
        nc.tensor.matmul(pw[:, 0:1], t_wint[:], x32[:, 1:2], start=True, stop=True)
        nc.scalar.activation(x32[:, 1:2], pw[:, 0:1], AF.Identity, bias=BIN, scale=1.0)
        # flag row fix: u = f*x0 + (1-f)*x999 ; w = Wout@u + bout ;
        # x0 += f*(w-u) ; x999 += (1-f)*(w-u)
        fcol = t_ff[:, i : i + 1]
        sm = np_.tile([D, 8], F32, tag="flagtmp")
        d1, u, t2, w_sb, d0 = (sm[:, j : j + 1] for j in range(5))
        nc.vector.tensor_tensor(d1, x32[:, 0:1], x32[:, 999:1000], ALU.subtract)
        nc.vector.tensor_scalar(d1, d1, fcol, None, ALU.mult)
        nc.vector.tensor_tensor(u, x32[:, 999:1000], d1, ALU.add)
        pf = ps.tile([P, 1024], F32, tag="ps")
        nc.tensor.matmul(pf[:, 0:1], t_woutt[:], u, start=True, stop=True)
        nc.scalar.activation(w_sb, pf[:, 0:1], AF.Identity, bias=BOUT, scale=1.0)
        nc.vector.tensor_tensor(t2, w_sb, u, ALU.subtract)
        nc.vector.tensor_scalar(d0, t2, fcol, None, ALU.mult)
        nc.vector.tensor_tensor(x32[:, 0:1], x32[:, 0:1], d0, ALU.add)
        nc.vector.tensor_tensor(x32[:, 999:1000], x32[:, 999:1000], t2, ALU.add)
        nc.vector.tensor_tensor(x32[:, 999:1000], x32[:, 999:1000], d0, ALU.subtract)
        x16 = xpool.tile([D, S], F16, tag=f"x16_{i}")
        nc.vector.tensor_copy(x16[:], x32[:])
        nc.vector.tensor_reduce(emb_mean[:, i : i + 1], x32[:], axis=mybir.AxisListType.X, op=ALU.add)
        x32s.append(x32)
        x16s.append(x16)
    nc.vector.tensor_scalar(emb_mean[:], emb_mean[:], 1.0 / S, None, ALU.mult)

    es_tiles = {}
    # carried per-item state: y2 tile of previous layer + (A2, C2) columns
    prev_y2 = list(x32s)      # layer 0: embedding tensor, A=1, C=0
    prev_AC = [None] * IPC

    # ---- encoder layers ----
    # Emission order == compile-time schedule order per engine, so phases are
    # emitted in pipeline order: all items' attention matmuls first, then the
    # per-group serial norm chains, then FFNs, so the PE always has queued
    # work while the DVE chains run.
    for l in range(L):
        ys, y2s, wgts, afts, sigs, rdens, pnums = {}, {}, {}, {}, {}, {}, {}
        sts = {}
        # ---- phase A: attention for all items ----
        for i in range(IPC):
            g0 = (i // GRP) * GRP
            j = i - g0
            if j == 0:
                sts[g0] = (np_.tile([D, 2, GRP], F32, tag=f"st1_{g0}", name=f"st1_{g0}"),
                           np_.tile([D, 2, GRP], F32, tag=f"st2_{g0}", name=f"st2_{g0}"))
            st1 = sts[g0][0]
            x16 = x16s[i]
            pq = ps.tile([P, 1024], F32, tag="ps")
            nc.tensor.matmul(pq[:, 0:500], t_wqt[l][:], x16[:, 0:500], start=True, stop=True)
            nc.tensor.matmul(pq[:, 512:1012], t_wqt[l][:], x16[:, 500:1000], start=True, stop=True)
            pk = ps.tile([P, 1024], F32, tag="ps")
            pv = ps.tile([P, 1024], F32, tag="ps")
            for c in range(TC):
                lhs = x16[:, c * TCS : (c + 1) * TCS]
                nc.tensor.matmul(pk[:TCS, c * P : (c + 1) * P], lhs, t_wkt[l][:], start=True, stop=True)
                nc.tensor.matmul(pv[:TCS, c * P : (c + 1) * P], lhs, t_wvt[l][:], start=True, stop=True)
            tq = tp.tile([P, S], F16, tag="tq")
            nc.scalar.activation(_v2(tq), _nv(pq), AF.Tanh, bias=0.0, scale=0.5)
            sig = pp.tile([P, S], F16, tag=f"sig_{i}")
            nc.gpsimd.tensor_scalar(sig[:], tq[:], 0.5, 0.5, ALU.mult, ALU.add)
            sigs[i] = sig
            ek = tp.tile([P, TC * P], BF16, tag="ek")
            nc.scalar.activation(ek[:TCS], pk[:TCS], AF.Exp, bias=0.0, scale=1.0)
            ekv = tp.tile([P, TC * P], BF16, tag="ekv")
            nc.vector.tensor_tensor(ekv[:TCS], ek[:TCS], pv[:TCS], ALU.mult)
            if shared_es:
                if l == 0:
                    es = singles.tile([P, TC, S], BF16, tag=f"es{i}")
                    nc.sync.dma_start(es[:TCS, 0:4], g["dist_t"][i, 0:4].rearrange("c p s -> p c s"))
                    nc.sync.dma_start(es[:TCS, 4:8], g["dist_t"][i, 4:8].rearrange("c p s -> p c s"))
                    nc.scalar.activation(es[:TCS, 0:4], es[:TCS, 0:4], AF.Exp, bias=0.0, scale=-cs[0])
                    nc.scalar.activation(es[:TCS, 4:8], es[:TCS, 4:8], AF.Exp, bias=0.0, scale=-cs[0])
                    es_tiles[i] = es
                es = es_tiles[i]
            else:
                es = tp.tile([P, TC, S], BF16, tag="es_dyn")
                nc.sync.dma_start(es[:TCS], g["dist_t"][i].rearrange("c p s -> p c s"))
                nc.scalar.activation(es[:TCS], es[:TCS], AF.Exp, bias=0.0, scale=-cs[l])
            pden = ps.tile([P, 1024], F32, tag="ps")
            for sc in range(SC):
                off = sc * 512
                ssl = slice(sc * SCS, (sc + 1) * SCS)
                for c in range(TC):
                    nc.tensor.matmul(pden[:, off : off + SCS],
                                     ek[:TCS, c * P : (c + 1) * P], es[:TCS, c, ssl],
                                     start=(c == 0), stop=(c == TC - 1))
            rden = scr.tile([P, 1024], F32, tag="rden")
            nc.vector.reciprocal_approx_fast(out=_nv(rden), in_=_nv(pden))
            rdens[i] = rden
            pnum = ps.tile([P, 1024], F32, tag="ps")
            for sc in range(SC):
                off = sc * 512
                ssl = slice(sc * SCS, (sc + 1) * SCS)
                for c in range(TC):
                    nc.tensor.matmul(pnum[:, off : off + SCS],
                                     ekv[:TCS, c * P : (c + 1) * P], es[:TCS, c, ssl],
                                     start=(c == 0), stop=(c == TC - 1))
            wgt = pp.tile([P, S], F16, tag=f"wgt_{i}")
            nc.vector.tensor_tensor(_v2(wgt), _nv(pnum), _nv(rden), ALU.mult)
            wgts[i] = wgt
        # ---- phase B: per-group norm-1 chains ----
        AC1 = {}
        for g0 in range(0, IPC, GRP):
            st1 = sts[g0][0]
            for i in range(g0, g0 + GRP):
                j = i - g0
                aft = tp.tile([P, S], F16, tag="tq", name="aft")
                nc.vector.affine_mul_reduce(
                    out=aft[:], accum_out=st1[:, 0, j : j + 1],
                    in0=wgts[i][:], in1=sigs[i][:], scale=1.0, bias=0.0)
                y = pp.tile([P, S], F32, tag=f"y_{i}")
                if prev_AC[i] is None:
                    nc.vector.tensor_tensor(y[:], prev_y2[i][:], aft[:], ALU.add)
                else:
                    A2p, C2p = prev_AC[i]
                    nc.vector.affine_then_add(out=y[:], in0=prev_y2[i][:], in1=aft[:],
                                              scale=A2p, bias=C2p)
                ys[i] = y
                sq = scr.tile([P, 1024], F32, tag="scr4k")
                nc.scalar.activation(sq[:, 0:S], y[:], AF.Square, accum_out=st1[:, 1, j : j + 1])
            mb = t_b2[:, l - 1 : l] if l > 0 else None
            mbc = emb_mean[:, g0 : g0 + GRP] if l == 0 else None
            A1, C1 = _norm_smalls(nc, np_, st1[:, 0], st1[:, 1],
                                  t_g1[:, l : l + 1], t_b1[:, l : l + 1], f"n1_{g0}",
                                  mean_bias=mb, mean_bias_cols=mbc)
            AC1[g0] = (A1, C1)
            for i in range(g0, g0 + GRP):
                j = i - g0
                h16 = pp.tile([P, S], F16, tag=f"h16_{i}")
                nc.gpsimd.tensor_scalar(h16[:], ys[i][:], A1[:, j : j + 1], C1[:, j : j + 1],
                                        ALU.mult, ALU.add)
                ys[f"h{i}"] = h16
        # ---- phases C+D interleaved per group: FFN then norm-2 ----
        for g0 in range(0, IPC, GRP):
            st2 = sts[g0][1]
            A1, C1 = AC1[g0]
            for i in range(g0, g0 + GRP):
                j = i - g0
                h16 = ys[f"h{i}"]
                pf2 = ps.tile([P, 1024], F32, tag="ps")
                for fc in range(FC):
                    pf1 = ps.tile([P, 1024], F32, tag="ps")
                    w1 = t_w1t[l][:, fc * P : (fc + 1) * P]
                    nc.tensor.matmul(pf1[:, 0:500], w1, h16[:, 0:500], start=True, stop=True)
                    nc.tensor.matmul(pf1[:, 512:1012], w1, h16[:, 500:1000], start=True, stop=True)
                    r16 = tp.tile([P, S], F16, tag="r16")
                    nc.scalar.activation(_v2(r16), _nv(pf1),
                                         AF.Relu, bias=t_bw1[:, l, fc : fc + 1], scale=1.0)
                    for sc in range(SC):
                        nc.tensor.matmul(pf2[:, sc * 512 : sc * 512 + SCS],
                                         t_w2t[l][:, fc, :], r16[:, sc * SCS : (sc + 1) * SCS],
                                         start=(fc == 0), stop=(fc == FC - 1))
                # y2 = h + ff = (A1*y + C1) + pf2   (bW2 cancels in next norm)
                y2 = pp.tile([P, S], F32, tag=f"y2_{i}")
                for sc in range(SC):
                    nc.vector.affine_then_add(
                        out=y2[:, sc * SCS : (sc + 1) * SCS],
                        in0=ys[i][:, sc * SCS : (sc + 1) * SCS],
                        in1=pf2[:, sc * 512 : sc * 512 + SCS],
                        scale=A1[:, j : j + 1], bias=C1[:, j : j + 1])
                y2s[i] = y2
                nc.vector.tensor_reduce(st2[:, 0, j : j + 1], y2[:], axis=mybir.AxisListType.X, op=ALU.add)
                sq = scr.tile([P, 1024], F32, tag="scr4k")
                nc.scalar.activation(sq[:, 0:S], y2[:], AF.Square, accum_out=st2[:, 1, j : j + 1])
            A2, C2 = _norm_smalls(nc, np_, st2[:, 0], st2[:, 1],
                                  t_g2[:, l : l + 1], t_b2[:, l : l + 1], f"n2_{g0}")
            for i in range(g0, g0 + GRP):
                j = i - g0
                if l < L - 1:
                    nx16 = xpool.tile([D, S], F16, tag=f"x16_{i}{'b' if l % 2 == 0 else ''}")
                    nc.gpsimd.tensor_scalar(nx16[:], y2s[i][:], A2[:, j : j + 1], C2[:, j : j + 1],
                                            ALU.mult, ALU.add)
                    x16s[i] = nx16
                else:
                    xout = xpool.tile([D, S], F32, tag=f"x32_{i}")
                    nc.gpsimd.tensor_scalar(xout[:], y2s[i][:], A2[:, j : j + 1], C2[:, j : j + 1],
                                            ALU.mult, ALU.add)
                    nc.sync.dma_start(g["out32"][i], xout[:])
                prev_y2[i] = y2s[i]
                prev_AC[i] = (A2[:, j : j + 1], C2[:, j : j + 1])

    ctx.close()


# ------------------------------------------------------------------
# host wrapper
# ------------------------------------------------------------------
_cache = {}


def _get_nc(cs_key):
    if cs_key not in _cache:
        _cache[cs_key] = build_cvrp(list(cs_key))
    return _cache[cs_key]


def prep_inputs(depot_xy, node_xy_demand, dist, log_scale, flag,
                Wd, bd, Wn, bn, Win, bin_, Wout, bout,
                Wq, Wk, Wv, alpha, g1, b1, W1, bW1, W2, bW2, g2, b2):
    import ml_dtypes

    flag = np.asarray(flag)
    cs = tuple(float(np.asarray(log_scale)[0]) * float(a) for a in np.asarray(alpha))

    dist_t = np.ascontiguousarray(np.asarray(dist).transpose(0, 2, 1)).astype(ml_dtypes.bfloat16)
    dist_t = dist_t.reshape(B, TC, TCS, S)
    node_t = np.ascontiguousarray(np.asarray(node_xy_demand).transpose(0, 2, 1)).astype(np.float16)
    depot = np.asarray(depot_xy).reshape(B, 2).astype(np.float32)
    flagf = flag.astype(np.float32)

    f16 = lambda a: np.ascontiguousarray(np.asarray(a)).astype(np.float16)
    f32 = lambda a: np.ascontiguousarray(np.asarray(a)).astype(np.float32)
    shared = {
        "wqt": f16(np.asarray(Wq).transpose(0, 2, 1)),
        "wkt": f16(np.asarray(Wk).transpose(0, 2, 1)),
        "wvt": f16(np.asarray(Wv).transpose(0, 2, 1)),
        "w1t": f16(np.asarray(W1).transpose(0, 2, 1)),
        "w2t": f16(np.asarray(W2).transpose(0, 2, 1).reshape(L, FC, P, D).transpose(0, 2, 1, 3)),
        "wnt": f16(np.asarray(Wn).T),
        "wdt": f32(np.asarray(Wd).T),
        "wint": f32(np.asarray(Win).T),
        "woutt": f32(np.asarray(Wout).T),
        "biases4": f32(np.stack([np.asarray(bd), np.asarray(bn),
                                 np.asarray(bin_), np.asarray(bout)], axis=1)),
        "bw1_t": f32(np.asarray(bW1).reshape(L, FC, P).transpose(2, 0, 1)),
        "g1_t": f32(np.asarray(g1).T),
        "b1_t": f32(np.asarray(b1).T),
        "g2_t": f32(np.asarray(g2).T),
        "b2_t": f32(np.asarray(b2).T),
    }
    in_maps = []
    for c in range(NCORES):
        sl = slice(c * IPC, (c + 1) * IPC)
        m = dict(shared)
        m["dist_t"] = dist_t[sl]
        m["node_t"] = node_t[sl]
        m["depot"] = depot[sl]
        m["flagf"] = flagf[sl]
        in_maps.append(m)
    return cs, in_maps


TRACE = False
LAST_RESULT = None


def kernel(**inputs):
    global LAST_RESULT
    cs, in_maps = prep_inputs(**inputs)
    nc = _get_nc(cs)
    res = run_bass_kernel_spmd(nc, in_maps, list(range(NCORES)), trace=TRACE)
    LAST_RESULT = res
    out = np.concatenate([r["out32"] for r in res.results], axis=0)  # [B, D, S]
    return np.ascontiguousarray(out.transpose(0, 2, 1)).astype(np.float32)



# revision 8
# speedup vs baseline: 1.1614x; 1.1614x over previous
"""Trainium2 Bass kernel for nn_CVRP_Encoder (AFT-style CVRP encoder).

Data-parallel over batch B=32 across 8 NeuronCores (4 items/core). Per item
everything lives in a transposed [D=128 (partitions), S=1000 (free)] layout so
instance-norm reduces along the free axis.

Key optimizations over the f16 version:
 - es = exp(-c*dist) is precomputed on the host and shipped as fp8-e4m3 in a
   DoubleRow pair-grouped layout (halves HBM traffic, removes the on-device
   exp over 8MB, and enables double-pumped matmuls).
 - ek = exp(k - 4) and ekv = ek*v are stored e5m2; the global exp-shift
   cancels exactly in num/den. Attention contraction runs in fp8 DoubleRow
   mode: 2 contraction rows per cycle -> half the PE time of bf16.
 - Residual sums for instance norm ride the residual-add ops
   (scalar_tensor_tensor accum), so no standalone reduce passes.
 - Engine balancing: sigmoid/exp/square/2 relus on ACT, recip/mults/2 relus
   on DVE, residual adds + norm-applies on GPSIMD, all matmuls fp8/f16 on PE.
"""
import sys

sys.path.insert(0, "/opt/trn_rl_repo")

import numpy as np

import concourse.bass as bass
import concourse.tile as tile
from concourse import bacc, mybir
from concourse.bass_utils import run_bass_kernel_spmd

F32 = mybir.dt.float32
F16 = mybir.dt.float16
BF16 = mybir.dt.bfloat16
F8E4 = mybir.dt.float8e4
F8E5 = mybir.dt.float8e5
I32 = mybir.dt.int32
AF = mybir.ActivationFunctionType
ALU = mybir.AluOpType
DR = mybir.MatmulPerfMode.DoubleRow

B, N, D, F, L = 32, 999, 128, 512, 6
S = N + 1
P = 128
NCORES = 8
IPC = B // NCORES
TC = 8             # t-chunks of 125 (partition dim of attention contraction)
TCS = S // TC      # 125
PC = 4             # DoubleRow pair-chunks (2 t-chunks each)
SC = 2
SCS = S // SC      # 500
FC = F // P        # 4
EPS = 1e-5
KSHIFT = 4.0       # ek = exp(k - KSHIFT); cancels in num/den
GRP = 2            # items per norm-batching group


def _bcast_dram(handle, n_part, idx, count):
    ap = handle[:]
    return bass.AP(tensor=ap.tensor, offset=idx, ap=[[0, n_part], [1, count]])


def _nv(t):
    """[P, 1024] tile/psum -> [P, 2, 500] strided view (skip 512-pad)."""
    return t[:].rearrange("p (n s) -> p n s", n=2)[:, :, 0:SCS]


def _v2(t):
    """[P, S] tile -> [P, 2, 500] view."""
    return t[:].rearrange("p (n s) -> p n s", n=2)


def build_cvrp(cs):
    """cs: per-layer scale constants c_l = log_scale * alpha[l]."""
    shared_es = all(abs(c - cs[0]) < 1e-30 for c in cs)
    n_es = 1 if shared_es else L

    nc = bacc.Bacc("TRN2", target_bir_lowering=False, debug=False,
                   num_devices=NCORES)

    g = {}
    g["es8"] = nc.declare_dram_parameter("es8", [n_es, IPC, TCS, TC, S], F8E4, isOutput=False)
    g["node_t"] = nc.declare_dram_parameter("node_t", [IPC, 3, N], F16, isOutput=False)
    g["depot"] = nc.declare_dram_parameter("depot", [IPC, 2], F32, isOutput=False)
    g["flagf"] = nc.declare_dram_parameter("flagf", [IPC], F32, isOutput=False)
    g["wqt"] = nc.declare_dram_parameter("wqt", [L, D, D], F16, isOutput=False)
    g["wkt"] = nc.declare_dram_parameter("wkt", [L, D, D], F16, isOutput=False)
    g["wvt"] = nc.declare_dram_parameter("wvt", [L, D, D], F16, isOutput=False)
    g["w1t"] = nc.declare_dram_parameter("w1t", [L, D, F], F16, isOutput=False)
    g["w2t"] = nc.declare_dram_parameter("w2t", [L, P, FC, D], F16, isOutput=False)
    g["wnt"] = nc.declare_dram_parameter("wnt", [3, D], F16, isOutput=False)
    g["wdt"] = nc.declare_dram_parameter("wdt", [2, D], F32, isOutput=False)
    g["wint"] = nc.declare_dram_parameter("wint", [D, D], F32, isOutput=False)
    g["woutt"] = nc.declare_dram_parameter("woutt", [D, D], F32, isOutput=False)
    g["biases4"] = nc.declare_dram_parameter("biases4", [D, 4], F32, isOutput=False)
    g["bw1_t"] = nc.declare_dram_parameter("bw1_t", [D, L, FC], F32, isOutput=False)
    g["g1_t"] = nc.declare_dram_parameter("g1_t", [D, L], F32, isOutput=False)
    g["b1_t"] = nc.declare_dram_parameter("b1_t", [D, L], F32, isOutput=False)
    g["g2_t"] = nc.declare_dram_parameter("g2_t", [D, L], F32, isOutput=False)
    g["b2_t"] = nc.declare_dram_parameter("b2_t", [D, L], F32, isOutput=False)
    g["out32"] = nc.declare_dram_parameter("out32", [IPC, D, S], F32, isOutput=True)

    with tile.TileContext(nc) as tc_ctx:
        _body(nc, tc_ctx, g, cs, shared_es)
    nc.compile()
    return nc


def _norm_smalls(nc, np_, sums, sumsq, g_col, b_col, eb, tag):
    """Instance-norm scalar math on [D, GRP] tiles.
    mean = sums/S; var = sumsq/S - mean^2; rstd = 1/sqrt(var + eps) via
    ACT Sqrt + DVE fast reciprocal. Returns (A, C): out = A*y + C."""
    sm = np_.tile([D, 8, GRP], F32, tag=f"nsm_{tag}")
    mean, msq, var, sd, ry, A, C = (sm[:, i] for i in range(7))
    nc.vector.tensor_scalar(mean, sums, 1.0 / S, None, ALU.mult)
    nc.vector.tensor_tensor(msq, mean, mean, ALU.mult)
    nc.vector.scalar_tensor_tensor(var, sumsq, 1.0 / S, msq, ALU.mult, ALU.subtract)
    nc.scalar.activation(sd, var, AF.Sqrt, bias=eb, scale=1.0)
    nc.vector.reciprocal_approx_fast(out=ry, in_=sd)
    nc.vector.tensor_scalar(A, ry, g_col, None, ALU.mult)
    nc.vector.tensor_tensor(C, mean, A, ALU.mult)
    nc.vector.tensor_scalar(C, C, b_col, -1.0, ALU.subtract, ALU.mult)
    return A, C


def _body(nc, tc, g, cs, shared_es):
    from contextlib import ExitStack

    ctx = ExitStack()
    singles = ctx.enter_context(tc.tile_pool(name="singles", bufs=1))
    xpool = ctx.enter_context(tc.tile_pool(name="xpool", bufs=1))
    tp = ctx.enter_context(tc.tile_pool(name="tp", bufs=2))
    scr = ctx.enter_context(tc.tile_pool(name="scr", bufs=2))
    np_ = ctx.enter_context(tc.tile_pool(name="npool", bufs=2))
    pp = ctx.enter_context(tc.tile_pool(name="pp", bufs=1))
    ps = ctx.enter_context(tc.tile_pool(name="ps", bufs=4, space="PSUM"))

    # ---- embedding weights + per-item inputs first (gate the pipeline) ----
    t_wnt = singles.tile([3, D], F16, tag="wnt")
    nc.sync.dma_start(t_wnt[:], g["wnt"][:])
    t_wdt = singles.tile([2, D], F32, tag="wdt")
    nc.sync.dma_start(t_wdt[:], g["wdt"][:])
    t_wint = singles.tile([D, D], F32, tag="wint")
    nc.sync.dma_start(t_wint[:], g["wint"][:])
    t_woutt = singles.tile([D, D], F32, tag="woutt")
    nc.sync.dma_start(t_woutt[:], g["woutt"][:])
    sm_t = {}
    for nm, shp in (("biases4", [D, 4]), ("bw1_t", [D, L, FC]), ("g1_t", [D, L]),
                    ("b1_t", [D, L]), ("g2_t", [D, L]), ("b2_t", [D, L])):
        t = singles.tile(shp, F32, tag=nm)
        nc.sync.dma_start(t[:], g[nm][:])
        sm_t[nm] = t
    t_b4, t_bw1 = sm_t["biases4"], sm_t["bw1_t"]
    t_g1, t_b1, t_g2, t_b2 = sm_t["g1_t"], sm_t["b1_t"], sm_t["g2_t"], sm_t["b2_t"]
    t_ff = singles.tile([P, IPC], F32, tag="ffl")
    nc.sync.dma_start(t_ff[:], _bcast_dram(g["flagf"], P, 0, IPC))
    t_const = singles.tile([P, 2], F32, tag="consts")
    nc.gpsimd.memset(t_const[:, 0:1], -KSHIFT)
    nc.gpsimd.memset(t_const[:, 1:2], EPS)
    KB, EB = t_const[:, 0:1], t_const[:, 1:2]

    # ---- layer-0 weights, then es8, then remaining layers ----
    t_wqt, t_wkt, t_wvt, t_w1t, t_w2t = [], [], [], [], []
    wlists = ((t_wqt, g["wqt"], [D, D]), (t_wkt, g["wkt"], [D, D]),
              (t_wvt, g["wvt"], [D, D]), (t_w1t, g["w1t"], [D, F]),
              (t_w2t, g["w2t"], [P, FC, D]))

    def load_layer_weights(l):
        for lst, src, shape in wlists:
            w = singles.tile(shape, F16, tag=f"w{id(lst)}_{l}", name=f"w_{l}")
            nc.sync.dma_start(w[:], src[l])
            lst.append(w)

    load_layer_weights(0)
    es_tiles = {}
    if shared_es:
        for i in range(IPC):
            es = singles.tile([TCS, TC, S], F8E4, tag=f"es{i}", name=f"es{i}")
            nc.sync.dma_start(es[:, 0:4], g["es8"][0, i, :, 0:4])
            nc.sync.dma_start(es[:, 4:8], g["es8"][0, i, :, 4:8])
            es_tiles[i] = es
    for l in range(1, L):
        load_layer_weights(l)

    BD, BN_, BIN, BOUT = (t_b4[:, i : i + 1] for i in range(4))

    # ---- embedding ----
    x16s = []
    for i in range(IPC):
        x32 = xpool.tile([D, S], F32, tag=f"x32_{i}", name=f"x32_{i}")
        t_node = scr.tile([P, 1024], F16, tag="node16")
        nc.sync.dma_start(t_node[:3, 0:N], g["node_t"][i])
        t_dep = tp.tile([2, 1], F32, tag="dep")
        nc.sync.dma_start(t_dep[:], g["depot"][i, :, None])
        pe = ps.tile([P, 1024], F32, tag="ps")
        nc.tensor.matmul(pe[:, 0:500], t_wnt[:], t_node[:3, 0:500], start=True, stop=True)
        nc.tensor.matmul(pe[:, 512:1011], t_wnt[:], t_node[:3, 500:999], start=True, stop=True)
        nc.scalar.activation(x32[:, 1:501], pe[:, 0:500], AF.Identity, bias=BN_, scale=1.0)
        nc.scalar.activation(x32[:, 501:1000], pe[:, 512:1011], AF.Identity, bias=BN_, scale=1.0)
        pd = ps.tile([P, 1024], F32, tag="ps")
        nc.tensor.matmul(pd[:, 0:1], t_wdt[:], t_dep[:], start=True, stop=True)
        nc.scalar.activation(x32[:, 0:1], pd[:, 0:1], AF.Identity, bias=BD, scale=1.0)
        pw = ps.tile([P, 1024], F32, tag="ps")
        nc.tensor.matmul(pw[:, 0:1], t_wint[:], x32[:, 1:2], start=True, stop=True)
        nc.scalar.activation(x32[:, 1:2], pw[:, 0:1], AF.Identity, bias=BIN, scale=1.0)
        # flag row fix: u = f*x0 + (1-f)*x999 ; w = Wout@u + bout ;
        # x0 += f*(w-u) ; x999 += (1-f)*(w-u)
        fcol = t_ff[:, i : i + 1]
        sm = np_.tile([D, 8], F32, tag="flagtmp")
        d1, u, t2, w_sb, d0 = (sm[:, j : j + 1] for j in range(5))
        nc.vector.tensor_tensor(d1, x32[:, 0:1], x32[:, 999:1000], ALU.subtract)
        nc.vector.tensor_scalar(d1, d1, fcol, None, ALU.mult)
        nc.vector.tensor_tensor(u, x32[:, 999:1000], d1, ALU.add)
        pf = ps.tile([P, 1024], F32, tag="ps")
        nc.tensor.matmul(pf[:, 0:1], t_woutt[:], u, start=True, stop=True)
        nc.scalar.activation(w_sb, pf[:, 0:1], AF.Identity, bias=BOUT, scale=1.0)
        nc.vector.tensor_tensor(t2, w_sb, u, ALU.subtract)
        nc.vector.tensor_scalar(d0, t2, fcol, None, ALU.mult)
        nc.vector.tensor_tensor(x32[:, 0:1], x32[:, 0:1], d0, ALU.add)
        nc.vector.tensor_tensor(x32[:, 999:1000], x32[:, 999:1000], t2, ALU.add)
        nc.vector.tensor_tensor(x32[:, 999:1000], x32[:, 999:1000], d0, ALU.subtract)
        x16 = xpool.tile([D, S], F16, tag=f"x16_{i}", name=f"x16_{i}")
        nc.vector.tensor_copy(x16[:], x32[:])
        x16s.append(x16)

    # ---- encoder layers ----
    for l in range(L):
        sigs, wgts, afts, ys, h16s, y2s = {}, {}, {}, {}, {}, {}
        sts = {}
        # ---- phase A: attention for all items ----
        for i in range(IPC):
            g0 = (i // GRP) * GRP
            if i == g0:
                sts[g0] = (np_.tile([D, 2, GRP], F32, tag=f"st1_{g0}", name=f"st1_{g0}"),
                           np_.tile([D, 2, GRP], F32, tag=f"st2_{g0}", name=f"st2_{g0}"))
            x16 = x16s[i]
            pq = ps.tile([P, 1024], F32, tag="ps")
            nc.tensor.matmul(pq[:, 0:500], t_wqt[l][:], x16[:, 0:500], start=True, stop=True)
            nc.tensor.matmul(pq[:, 512:1012], t_wqt[l][:], x16[:, 500:1000], start=True, stop=True)
            pk = ps.tile([P, 1024], F32, tag="ps")
            pv = ps.tile([P, 1024], F32, tag="ps")
            for c in range(TC):
                lhs = x16[:, c * TCS : (c + 1) * TCS]
                nc.tensor.matmul(pk[:TCS, c * P : (c + 1) * P], lhs, t_wkt[l][:], start=True, stop=True)
                nc.tensor.matmul(pv[:TCS, c * P : (c + 1) * P], lhs, t_wvt[l][:], start=True, stop=True)
            sig = pp.tile([P, S], F16, tag=f"sig_{i}", name=f"sig_{i}")
            nc.scalar.activation(_v2(sig), _nv(pq), AF.Sigmoid, bias=0.0, scale=1.0)
            sigs[i] = sig
            ek = tp.tile([P, TC * P], F8E5, tag="ek")
            nc.scalar.activation(ek[:TCS], pk[:TCS], AF.Exp, bias=KB[:TCS], scale=1.0)
            ekv = tp.tile([P, TC * P], F8E5, tag="ekv")
            nc.vector.tensor_tensor(ekv[:TCS], ek[:TCS], pv[:TCS], ALU.mult)
            if shared_es:
                es = es_tiles[i]
            else:
                es = tp.tile([TCS, TC, S], F8E4, tag="es_dyn")
                nc.sync.dma_start(es[:], g["es8"][l, i])
            pden = ps.tile([P, 1024], F32, tag="ps")
            for sc in range(SC):
                off = sc * 512
                ssl = slice(sc * SCS, (sc + 1) * SCS)
                for c in range(PC):
                    nc.tensor.matmul(pden[:, off : off + SCS],
                                     ek[:TCS, c * 2 * P : (c + 1) * 2 * P].rearrange("p (j d) -> p j d", j=2),
                                     es[:, 2 * c : 2 * c + 2, ssl],
                                     start=(c == 0), stop=(c == PC - 1), perf_mode=DR)
            rden = scr.tile([P, 1024], F32, tag="rden")
            nc.vector.reciprocal_approx_fast(out=_nv(rden), in_=_nv(pden))
            pnum = ps.tile([P, 1024], F32, tag="ps")
            for sc in range(SC):
                off = sc * 512
                ssl = slice(sc * SCS, (sc + 1) * SCS)
                for c in range(PC):
                    nc.tensor.matmul(pnum[:, off : off + SCS],
                                     ekv[:TCS, c * 2 * P : (c + 1) * 2 * P].rearrange("p (j d) -> p j d", j=2),
                                     es[:, 2 * c : 2 * c + 2, ssl],
                                     start=(c == 0), stop=(c == PC - 1), perf_mode=DR)
            wgt = pp.tile([P, S], F16, tag=f"wgt_{i}", name=f"wgt_{i}")
            nc.vector.tensor_tensor(_v2(wgt), _nv(pnum), _nv(rden), ALU.mult)
            wgts[i] = wgt
            aft = pp.tile([P, S], F16, tag=f"aft_{i}", name=f"aft_{i}")
            nc.gpsimd.tensor_tensor(aft[:], wgt[:], sig[:], ALU.mult)
            afts[i] = aft
        # ---- phase B: per-group norm-1 ----
        AC1 = {}
        for g0 in range(0, IPC, GRP):
            st1 = sts[g0][0]
            for i in range(g0, g0 + GRP):
                j = i - g0
                y = xpool.tile([P, S], F32, tag=f"y_{i}", name=f"y_{i}")
                nc.vector.scalar_tensor_tensor(y[:], x16s[i][:], 0.0, afts[i][:],
                                               ALU.add, ALU.add,
                                               accum_out=st1[:, 0, j : j + 1])
                ys[i] = y
                sq = scr.tile([P, 1024], F32, tag="sqscr")
                nc.scalar.activation(sq[:, 0:S], y[:], AF.Square, accum_out=st1[:, 1, j : j + 1])
            A1, C1 = _norm_smalls(nc, np_, st1[:, 0], st1[:, 1],
                                  t_g1[:, l : l + 1], t_b1[:, l : l + 1], EB, f"n1_{g0}")
            AC1[g0] = (A1, C1)
            for i in range(g0, g0 + GRP):
                j = i - g0
                h16 = pp.tile([P, S], F16, tag=f"h16_{i}", name=f"h16_{i}")
                nc.gpsimd.tensor_scalar(h16[:], ys[i][:], A1[:, j : j + 1], C1[:, j : j + 1],
                                        ALU.mult, ALU.add)
                h16s[i] = h16
        # ---- phases C+D per group: FFN then norm-2 ----
        for g0 in range(0, IPC, GRP):
            st2 = sts[g0][1]
            for i in range(g0, g0 + GRP):
                j = i - g0
                h16 = h16s[i]
                pf2 = ps.tile([P, 1024], F32, tag="ps")
                for fc in range(FC):
                    pf1 = ps.tile([P, 1024], F32, tag="ps")
                    w1 = t_w1t[l][:, fc * P : (fc + 1) * P]
                    nc.tensor.matmul(pf1[:, 0:500], w1, h16[:, 0:500], start=True, stop=True)
                    nc.tensor.matmul(pf1[:, 512:1012], w1, h16[:, 500:1000], start=True, stop=True)
                    r16 = tp.tile([P, S], F16, tag="r16")
                    bcol = t_bw1[:, l, fc : fc + 1]
                    if fc % 2 == 0:
                        nc.scalar.activation(_v2(r16), _nv(pf1), AF.Relu, bias=bcol, scale=1.0)
                    else:
                        nc.vector.tensor_scalar(_v2(r16), _nv(pf1), bcol, 0.0, ALU.add, ALU.max)
                    for sc in range(SC):
                        nc.tensor.matmul(pf2[:, sc * 512 : sc * 512 + SCS],
                                         t_w2t[l][:, fc, :], r16[:, sc * SCS : (sc + 1) * SCS],
                                         start=(fc == 0), stop=(fc == FC - 1))
                # y2 = h + ff  (bW2 cancels in next norm)
                y2 = xpool.tile([P, S], F32, tag=f"y2_{i}", name=f"y2_{i}")
                nc.vector.scalar_tensor_tensor(_v2(y2), _v2(h16), 0.0, _nv(pf2),
                                               ALU.add, ALU.add,
                                               accum_out=st2[:, 0, j : j + 1])
                y2s[i] = y2
                sq = scr.tile([P, 1024], F32, tag="sqscr")
                nc.scalar.activation(sq[:, 0:S], y2[:], AF.Square, accum_out=st2[:, 1, j : j + 1])
            A2, C2 = _norm_smalls(nc, np_, st2[:, 0], st2[:, 1],
                                  t_g2[:, l : l + 1], t_b2[:, l : l + 1], EB, f"n2_{g0}")
            for i in range(g0, g0 + GRP):
                j = i - g0
                if l < L - 1:
                    nx16 = xpool.tile([D, S], F16, tag=f"x16_{i}{'b' if l % 2 == 0 else ''}",
                                      name=f"nx16_{i}")
                    nc.gpsimd.tensor_scalar(nx16[:], y2s[i][:], A2[:, j : j + 1], C2[:, j : j + 1],
                                            ALU.mult, ALU.add)
                    x16s[i] = nx16
                else:
                    xout = xpool.tile([D, S], F32, tag=f"x32_{i}", name=f"xout_{i}")
                    nc.gpsimd.tensor_scalar(xout[:], y2s[i][:], A2[:, j : j + 1], C2[:, j : j + 1],
                                            ALU.mult, ALU.add)
                    nc.sync.dma_start(g["out32"][i], xout[:])

    ctx.close()


# ------------------------------------------------------------------
# host wrapper
# ------------------------------------------------------------------
_cache = {}


def _get_nc(cs_key):
    if cs_key not in _cache:
        _cache[cs_key] = build_cvrp(list(cs_key))
    return _cache[cs_key]


def prep_inputs(depot_xy, node_xy_demand, dist, log_scale, flag,
                Wd, bd, Wn, bn, Win, bin_, Wout, bout,
                Wq, Wk, Wv, alpha, g1, b1, W1, bW1, W2, bW2, g2, b2):
    import ml_dtypes

    flag = np.asarray(flag)
    cs = tuple(float(np.asarray(log_scale)[0]) * float(a) for a in np.asarray(alpha))
    shared_es = all(abs(c - cs[0]) < 1e-30 for c in cs)

    # es8[b, p, cc, s] = exp(-c_l * dist[b, s, cc*125+p]), fp8-e4m3,
    # cc = consecutive 125-row t-chunks (DoubleRow pairs are (2c, 2c+1)).
    dist_t = np.asarray(dist).transpose(0, 2, 1).reshape(B, TC, TCS, S)
    layers = [cs[0]] if shared_es else list(cs)
    es8 = np.empty((len(layers), B, TCS, TC, S), dtype=ml_dtypes.float8_e4m3)
    for li, c in enumerate(layers):
        es8[li] = np.exp(-c * dist_t).transpose(0, 2, 1, 3).astype(ml_dtypes.float8_e4m3)

    node_t = np.ascontiguousarray(np.asarray(node_xy_demand).transpose(0, 2, 1)).astype(np.float16)
    depot = np.asarray(depot_xy).reshape(B, 2).astype(np.float32)
    flagf = flag.astype(np.float32)

    f16 = lambda a: np.ascontiguousarray(np.asarray(a)).astype(np.float16)
    f32 = lambda a: np.ascontiguousarray(np.asarray(a)).astype(np.float32)
    shared = {
        "wqt": f16(np.asarray(Wq).transpose(0, 2, 1)),
        "wkt": f16(np.asarray(Wk).transpose(0, 2, 1)),
        "wvt": f16(np.asarray(Wv).transpose(0, 2, 1)),
        "w1t": f16(np.asarray(W1).transpose(0, 2, 1)),
        "w2t": f16(np.asarray(W2).transpose(0, 2, 1).reshape(L, FC, P, D).transpose(0, 2, 1, 3)),
        "wnt": f16(np.asarray(Wn).T),
        "wdt": f32(np.asarray(Wd).T),
        "wint": f32(np.asarray(Win).T),
        "woutt": f32(np.asarray(Wout).T),
        "biases4": f32(np.stack([np.asarray(bd), np.asarray(bn),
                                 np.asarray(bin_), np.asarray(bout)], axis=1)),
        "bw1_t": f32(np.asarray(bW1).reshape(L, FC, P).transpose(2, 0, 1)),
        "g1_t": f32(np.asarray(g1).T),
        "b1_t": f32(np.asarray(b1).T),
        "g2_t": f32(np.asarray(g2).T),
        "b2_t": f32(np.asarray(b2).T),
    }
    in_maps = []
    for c in range(NCORES):
        sl = slice(c * IPC, (c + 1) * IPC)
        m = dict(shared)
        m["es8"] = np.ascontiguousarray(es8[:, sl])
        m["node_t"] = node_t[sl]
        m["depot"] = depot[sl]
        m["flagf"] = flagf[sl]
        in_maps.append(m)
    return cs, in_maps


TRACE = False
LAST_RESULT = None


def kernel(**inputs):
    global LAST_RESULT
    cs, in_maps = prep_inputs(**inputs)
    nc = _get_nc(cs)
    res = run_bass_kernel_spmd(nc, in_maps, list(range(NCORES)), trace=TRACE)
    LAST_RESULT = res
    out = np.concatenate([r["out32"] for r in res.results], axis=0)  # [B, D, S]
    return np.ascontiguousarray(out.transpose(0, 2, 1)).astype(np.float32)


# revision 11
# speedup vs baseline: 1.2908x; 1.1114x over previous
"""Trainium2 Bass kernel for nn_CVRP_Encoder (AFT-style CVRP encoder).

Data-parallel over batch B=32 across 8 NeuronCores (4 items/core). Per item
everything lives in a transposed [D=128 (partitions), S=1000 (free)] layout so
instance-norm reduces along the free axis.

Main design points:
 - es = exp(-c*dist) precomputed on host, shipped fp8-e4m3 in a DoubleRow
   pair-grouped layout; ek = exp(k-4), ekv = ek*v in e5m2 (the global exp
   shift cancels in num/den). Attention contraction runs double-pumped fp8:
   2 contraction rows/cycle.
 - ACT function mix stays inside one table set (exp_and_others: tanh, exp,
   square, relu, identity) so there are no ACT_TABLE_LOAD switches.
 - sigmoid(q)*wgt is one DVE affine_mul_reduce: (0.5*tanh(q/2)+0.5)*wgt,
   which also emits sum(aft) for the norm-1 mean (analytic residual mean).
 - k and v share one stationary pass: rhs = [Wk.T | Wv.T] (256 wide), halving
   k/v LDWEIGHTS traffic.
 - norm scalar math (magic-rsqrt + 2 Newton) runs on GPSIMD so DVE/ACT stay
   free; norm-apply also on GPSIMD -> A/C never cross engines.
 - Per-layer weight blob = one DMA per layer.
"""
import sys

sys.path.insert(0, "/opt/trn_rl_repo")

import numpy as np

import concourse.bass as bass
import concourse.tile as tile
from concourse import bacc, mybir
from concourse.bass_utils import run_bass_kernel_spmd

F32 = mybir.dt.float32
F16 = mybir.dt.float16
BF16 = mybir.dt.bfloat16
F8E4 = mybir.dt.float8e4
F8E5 = mybir.dt.float8e5
I32 = mybir.dt.int32
AF = mybir.ActivationFunctionType
ALU = mybir.AluOpType
DR = mybir.MatmulPerfMode.DoubleRow

B, N, D, F, L = 32, 999, 128, 512, 6
S = N + 1
P = 128
NCORES = 8
IPC = B // NCORES
TC = 8             # t-chunks of 125 (partition dim of attention contraction)
TCS = S // TC      # 125
PC = 4             # DoubleRow pair-chunks (2 t-chunks each)
SC = 2
SCS = S // SC      # 500
FC = F // P        # 4
EPS = 1e-5
KSHIFT = 4.0       # ek = exp(k - KSHIFT); cancels in num/den
GRP = 2            # items per norm-batching group
RSQRT_MAGIC = 0x5F3759DF + 1
WCOLS = 3 * D + 2 * F          # per-layer weight blob columns (1408)


def _bcast_dram(handle, n_part, idx, count):
    ap = handle[:]
    return bass.AP(tensor=ap.tensor, offset=idx, ap=[[0, n_part], [1, count]])


def _nv(t):
    """[P, 1024] tile/psum -> [P, 2, 500] strided view (skip 512-pad)."""
    return t[:].rearrange("p (n s) -> p n s", n=2)[:, :, 0:SCS]


def _v2(t):
    """[P, S] tile -> [P, 2, 500] view."""
    return t[:].rearrange("p (n s) -> p n s", n=2)


def build_cvrp(cs):
    """cs: per-layer scale constants c_l = log_scale * alpha[l]."""
    shared_es = all(abs(c - cs[0]) < 1e-30 for c in cs)
    n_es = 1 if shared_es else L

    nc = bacc.Bacc("TRN2", target_bir_lowering=False, debug=False,
                   num_devices=NCORES)

    g = {}
    g["es8"] = nc.declare_dram_parameter("es8", [n_es, IPC, TCS, TC, S], F8E4, isOutput=False)
    g["node_t"] = nc.declare_dram_parameter("node_t", [IPC, 3, N], F16, isOutput=False)
    g["depot"] = nc.declare_dram_parameter("depot", [IPC, 2], F32, isOutput=False)
    g["flagf"] = nc.declare_dram_parameter("flagf", [IPC], F32, isOutput=False)
    g["wblob"] = nc.declare_dram_parameter("wblob", [L, D, WCOLS], F16, isOutput=False)
    g["wnt"] = nc.declare_dram_parameter("wnt", [3, D], F16, isOutput=False)
    g["wdt"] = nc.declare_dram_parameter("wdt", [2, D], F32, isOutput=False)
    g["wint"] = nc.declare_dram_parameter("wint", [D, D], F32, isOutput=False)
    g["woutt"] = nc.declare_dram_parameter("woutt", [D, D], F32, isOutput=False)
    # sblob: biases4(4) | bw1 (L*FC=24) | g1(6) | b1(6) | g2(6) | b2(6) = 52
    g["sblob"] = nc.declare_dram_parameter("sblob", [D, 52], F32, isOutput=False)
    g["out32"] = nc.declare_dram_parameter("out32", [IPC, D, S], F32, isOutput=True)

    with tile.TileContext(nc) as tc_ctx:
        _body(nc, tc_ctx, g, cs, shared_es)
    nc.compile()
    return nc


def _norm_smalls(nc, np_, sums, sumsq, g_col, b_col, tag, mean_bias=None,
                 mean_bias_cols=None):
    """Instance-norm scalar math on [D, GRP] tiles, on GPSIMD.
    mean = sums/S (+bias); var = sumsq/S + eps - mean^2; rstd via magic
    rsqrt + 2 Newton iters. Returns (A, C): out = A*y + C."""
    e = nc.gpsimd
    sm = np_.tile([D, 8, GRP], F32, tag=f"nsm_{tag}")
    mean, msq, var = sm[:, 0], sm[:, 1], sm[:, 2]
    if mean_bias is not None:
        e.tensor_scalar(mean, sums, 1.0 / S, mean_bias, ALU.mult, ALU.add)
    else:
        e.tensor_scalar(mean, sums, 1.0 / S, None, ALU.mult)
    if mean_bias_cols is not None:
        e.tensor_tensor(mean, mean, mean_bias_cols, ALU.add)
    e.tensor_tensor(msq, mean, mean, ALU.mult)
    e.tensor_scalar(var, sumsq, 1.0 / S, EPS, ALU.mult, ALU.add)
    e.tensor_tensor(var, var, msq, ALU.subtract)
    ry = sm[:, 3]
    ibits = ry.bitcast(I32)
    # int bit-trick ops are not supported on Pool; run them on DVE
    nc.vector.tensor_scalar(ibits, var.bitcast(I32), 1, -1,
                            ALU.logical_shift_right, ALU.bitwise_xor)
    nc.vector.tensor_scalar(ibits, ibits, RSQRT_MAGIC, None, ALU.add)
    t1, t2 = sm[:, 4], sm[:, 5]
    for _ in range(2):
        e.tensor_tensor(t1, ry, ry, ALU.mult)
        e.tensor_tensor(t2, t1, var, ALU.mult)
        e.tensor_scalar(t2, t2, -0.5, 1.5, ALU.mult, ALU.add)
        e.tensor_tensor(ry, ry, t2, ALU.mult)
    A, C = sm[:, 6], sm[:, 7]
    e.tensor_scalar(A, ry, g_col, None, ALU.mult)
    e.tensor_tensor(C, mean, A, ALU.mult)
    e.tensor_scalar(C, C, b_col, -1.0, ALU.subtract, ALU.mult)
    return A, C


def _body(nc, tc, g, cs, shared_es):
    from contextlib import ExitStack

    ctx = ExitStack()
    singles = ctx.enter_context(tc.tile_pool(name="singles", bufs=1))
    xpool = ctx.enter_context(tc.tile_pool(name="xpool", bufs=1))
    tp = ctx.enter_context(tc.tile_pool(name="tp", bufs=2))
    scr = ctx.enter_context(tc.tile_pool(name="scr", bufs=2))
    np_ = ctx.enter_context(tc.tile_pool(name="npool", bufs=2))
    pp = ctx.enter_context(tc.tile_pool(name="pp", bufs=1))
    ps = ctx.enter_context(tc.tile_pool(name="ps", bufs=4, space="PSUM"))

    # ---- embedding weights + per-item inputs first (gate the pipeline) ----
    t_wnt = singles.tile([3, D], F16, tag="wnt")
    nc.sync.dma_start(t_wnt[:], g["wnt"][:])
    t_wdt = singles.tile([2, D], F32, tag="wdt")
    nc.sync.dma_start(t_wdt[:], g["wdt"][:])
    t_wint = singles.tile([D, D], F32, tag="wint")
    nc.sync.dma_start(t_wint[:], g["wint"][:])
    t_woutt = singles.tile([D, D], F32, tag="woutt")
    nc.sync.dma_start(t_woutt[:], g["woutt"][:])
    t_sb = singles.tile([D, 52], F32, tag="sblob")
    nc.sync.dma_start(t_sb[:], g["sblob"][:])
    t_b4 = t_sb[:, 0:4]
    t_bw1 = t_sb[:, 4:28].rearrange("p (l f) -> p l f", l=L)
    t_g1, t_b1 = t_sb[:, 28:34], t_sb[:, 34:40]
    t_g2, t_b2 = t_sb[:, 40:46], t_sb[:, 46:52]
    t_ff = singles.tile([P, IPC], F32, tag="ffl")
    nc.sync.dma_start(t_ff[:], _bcast_dram(g["flagf"], P, 0, IPC))
    t_const = singles.tile([P, 1], F32, tag="consts")
    nc.gpsimd.memset(t_const[:, 0:1], -KSHIFT)
    KB = t_const[:, 0:1]

    # ---- layer-0 weights, then es8, then remaining layers ----
    t_wb = []

    def load_layer_weights(l):
        w = singles.tile([D, WCOLS], F16, tag=f"wb_{l}", name=f"wb_{l}")
        nc.sync.dma_start(w[:], g["wblob"][l])
        t_wb.append(w)

    load_layer_weights(0)
    es_tiles = {}
    if shared_es:
        for i in range(IPC):
            es = singles.tile([TCS, TC, S], F8E4, tag=f"es{i}", name=f"es{i}")
            nc.sync.dma_start(es[:, 0:4], g["es8"][0, i, :, 0:4])
            nc.sync.dma_start(es[:, 4:8], g["es8"][0, i, :, 4:8])
            es_tiles[i] = es
    for l in range(1, L):
        load_layer_weights(l)

    def wq(l):
        return t_wb[l][:, 0:D]

    def wkv(l):
        return t_wb[l][:, D : 3 * D]

    def w1(l, fc):
        return t_wb[l][:, 3 * D + fc * P : 3 * D + (fc + 1) * P]

    def w2(l, fc):
        return t_wb[l][:, 3 * D + F + fc * P : 3 * D + F + (fc + 1) * P]

    BD, BN_, BIN, BOUT = (t_b4[:, i : i + 1] for i in range(4))
    emb_mean = singles.tile([D, IPC], F32, tag="embm")

    # ---- embedding ----
    x16s = []
    for i in range(IPC):
        x32 = xpool.tile([D, S], F32, tag=f"x32_{i}", name=f"x32_{i}")
        t_node = scr.tile([P, 1024], F16, tag="node16")
        nc.sync.dma_start(t_node[:3, 0:N], g["node_t"][i])
        t_dep = tp.tile([2, 1], F32, tag="dep")
        nc.sync.dma_start(t_dep[:], g["depot"][i, :, None])
        pe = ps.tile([P, 1024], F32, tag="ps")
        nc.tensor.matmul(pe[:, 0:500], t_wnt[:], t_node[:3, 0:500], start=True, stop=True)
        nc.tensor.matmul(pe[:, 512:1011], t_wnt[:], t_node[:3, 500:999], start=True, stop=True)
        nc.scalar.activation(x32[:, 1:501], pe[:, 0:500], AF.Identity, bias=BN_, scale=1.0)
        nc.scalar.activation(x32[:, 501:1000], pe[:, 512:1011], AF.Identity, bias=BN_, scale=1.0)
        pd = ps.tile([P, 1024], F32, tag="ps")
        nc.tensor.matmul(pd[:, 0:1], t_wdt[:], t_dep[:], start=True, stop=True)
        nc.scalar.activation(x32[:, 0:1], pd[:, 0:1], AF.Identity, bias=BD, scale=1.0)
        pw = ps.tile([P, 1024], F32, tag="ps")
        nc.tensor.matmul(pw[:, 0:1], t_wint[:], x32[:, 1:2], start=True, stop=True)
        nc.scalar.activation(x32[:, 1:2], pw[:, 0:1], AF.Identity, bias=BIN, scale=1.0)
        # flag row fix: u = f*x0 + (1-f)*x999 ; w = Wout@u + bout ;
        # x0 += f*(w-u) ; x999 += (1-f)*(w-u)
        fcol = t_ff[:, i : i + 1]
        sm = np_.tile([D, 8], F32, tag="flagtmp")
        d1, u, t2, w_sb, d0 = (sm[:, j : j + 1] for j in range(5))
        nc.vector.tensor_tensor(d1, x32[:, 0:1], x32[:, 999:1000], ALU.subtract)
        nc.vector.tensor_scalar(d1, d1, fcol, None, ALU.mult)
        nc.vector.tensor_tensor(u, x32[:, 999:1000], d1, ALU.add)
        pf = ps.tile([P, 1024], F32, tag="ps")
        nc.tensor.matmul(pf[:, 0:1], t_woutt[:], u, start=True, stop=True)
        nc.scalar.activation(w_sb, pf[:, 0:1], AF.Identity, bias=BOUT, scale=1.0)
        nc.vector.tensor_tensor(t2, w_sb, u, ALU.subtract)
        nc.vector.tensor_scalar(d0, t2, fcol, None, ALU.mult)
        nc.vector.tensor_tensor(x32[:, 0:1], x32[:, 0:1], d0, ALU.add)
        nc.vector.tensor_tensor(x32[:, 999:1000], x32[:, 999:1000], t2, ALU.add)
        nc.vector.tensor_tensor(x32[:, 999:1000], x32[:, 999:1000], d0, ALU.subtract)
        x16 = xpool.tile([D, S], F16, tag=f"x16_{i}", name=f"x16_{i}")
        nc.vector.tensor_copy(x16[:], x32[:])
        nc.vector.tensor_reduce(emb_mean[:, i : i + 1], x32[:], axis=mybir.AxisListType.X, op=ALU.add)
        x16s.append(x16)
    nc.vector.tensor_scalar(emb_mean[:], emb_mean[:], 1.0 / S, None, ALU.mult)

    # ---- encoder layers ----
    for l in range(L):
        wgts, afts, ys, h16s, y2s = {}, {}, {}, {}, {}
        sts = {}
        # ---- phase A: attention for all items ----
        for i in range(IPC):
            g0 = (i // GRP) * GRP
            if i == g0:
                sts[g0] = (np_.tile([D, 2, GRP], F32, tag=f"st1_{g0}", name=f"st1_{g0}"),
                           np_.tile([D, 2, GRP], F32, tag=f"st2_{g0}", name=f"st2_{g0}"))
            x16 = x16s[i]
            pq = ps.tile([P, 1024], F32, tag="ps")
            nc.tensor.matmul(pq[:, 0:500], wq(l), x16[:, 0:500], start=True, stop=True)
            nc.tensor.matmul(pq[:, 512:1012], wq(l), x16[:, 500:1000], start=True, stop=True)
            pkv_a = ps.tile([P, 1024], F32, tag="ps")
            pkv_b = ps.tile([P, 1024], F32, tag="ps")
            for c in range(TC):
                pkv = pkv_a if c < 4 else pkv_b
                cc = c % 4
                nc.tensor.matmul(pkv[:TCS, cc * 256 : (cc + 1) * 256],
                                 x16[:, c * TCS : (c + 1) * TCS], wkv(l),
                                 start=True, stop=True)
            tq = tp.tile([P, S], F16, tag="tq")
            nc.scalar.activation(_v2(tq), _nv(pq), AF.Tanh, bias=0.0, scale=0.5)
            ek = tp.tile([P, TC * P], F8E5, tag="ek")
            ekv = tp.tile([P, TC * P], F8E5, tag="ekv")
            for h, pkv in ((0, pkv_a), (1, pkv_b)):
                pv4 = pkv[:TCS].rearrange("p (c two d) -> p c two d", two=2, d=P)
                eko = ek[:TCS, h * 512 : (h + 1) * 512].rearrange("p (c d) -> p c d", d=P)
                ekvo = ekv[:TCS, h * 512 : (h + 1) * 512].rearrange("p (c d) -> p c d", d=P)
                nc.scalar.activation(eko, pv4[:, :, 0], AF.Exp, bias=KB[:TCS], scale=1.0)
                nc.vector.tensor_tensor(ekvo, eko, pv4[:, :, 1], ALU.mult)
            if shared_es:
                es = es_tiles[i]
            else:
                es = tp.tile([TCS, TC, S], F8E4, tag="es_dyn")
                nc.sync.dma_start(es[:], g["es8"][l, i])
            pden = ps.tile([P, 1024], F32, tag="ps")
            for sc in range(SC):
                off = sc * 512
                ssl = slice(sc * SCS, (sc + 1) * SCS)
                for c in range(PC):
                    nc.tensor.matmul(pden[:, off : off + SCS],
                                     ek[:TCS, c * 2 * P : (c + 1) * 2 * P].rearrange("p (j d) -> p j d", j=2),
                                     es[:, 2 * c : 2 * c + 2, ssl],
                                     start=(c == 0), stop=(c == PC - 1), perf_mode=DR)
            rden = scr.tile([P, 1024], F32, tag="rden")
            nc.vector.reciprocal_approx_fast(out=_nv(rden), in_=_nv(pden))
            pnum = ps.tile([P, 1024], F32, tag="ps")
            for sc in range(SC):
                off = sc * 512
                ssl = slice(sc * SCS, (sc + 1) * SCS)
                for c in range(PC):
                    nc.tensor.matmul(pnum[:, off : off + SCS],
                                     ekv[:TCS, c * 2 * P : (c + 1) * 2 * P].rearrange("p (j d) -> p j d", j=2),
                                     es[:, 2 * c : 2 * c + 2, ssl],
                                     start=(c == 0), stop=(c == PC - 1), perf_mode=DR)
            wgt = pp.tile([P, S], F16, tag=f"wgt_{i}", name=f"wgt_{i}")
            nc.vector.tensor_tensor(_v2(wgt), _nv(pnum), _nv(rden), ALU.mult)
            wgts[i] = wgt
            st1 = sts[g0][0]
            aft = pp.tile([P, S], F16, tag=f"aft_{i}", name=f"aft_{i}")
            nc.vector.affine_mul_reduce(out=aft[:], accum_out=st1[:, 0, (i - g0) : (i - g0) + 1],
                                        in0=tq[:], in1=wgt[:], scale=0.5, bias=0.5)
            afts[i] = aft
        # ---- phase B: per-group norm-1 ----
        for g0 in range(0, IPC, GRP):
            st1 = sts[g0][0]
            for i in range(g0, g0 + GRP):
                j = i - g0
                y = xpool.tile([P, S], F32, tag=f"y_{i}", name=f"y_{i}")
                nc.vector.tensor_tensor(y[:], x16s[i][:], afts[i][:], ALU.add)
                ys[i] = y
                sq = scr.tile([P, 1024], F32, tag="sqscr")
                nc.scalar.activation(sq[:, 0:S], y[:], AF.Square, accum_out=st1[:, 1, j : j + 1])
            mb = t_b2[:, l - 1 : l] if l > 0 else None
            mbc = emb_mean[:, g0 : g0 + GRP] if l == 0 else None
            A1, C1 = _norm_smalls(nc, np_, st1[:, 0], st1[:, 1],
                                  t_g1[:, l : l + 1], t_b1[:, l : l + 1], f"n1_{g0}",
                                  mean_bias=mb, mean_bias_cols=mbc)
            for i in range(g0, g0 + GRP):
                j = i - g0
                h16 = pp.tile([P, S], F16, tag=f"h16_{i}", name=f"h16_{i}")
                nc.gpsimd.tensor_scalar(h16[:], ys[i][:], A1[:, j : j + 1], C1[:, j : j + 1],
                                        ALU.mult, ALU.add)
                h16s[i] = h16
        # ---- phases C+D per group: FFN then norm-2 ----
        for g0 in range(0, IPC, GRP):
            st2 = sts[g0][1]
            for i in range(g0, g0 + GRP):
                j = i - g0
                h16 = h16s[i]
                pf2 = ps.tile([P, 1024], F32, tag="ps")
                for fc in range(FC):
                    pf1 = ps.tile([P, 1024], F32, tag="ps")
                    nc.tensor.matmul(pf1[:, 0:500], w1(l, fc), h16[:, 0:500], start=True, stop=True)
                    nc.tensor.matmul(pf1[:, 512:1012], w1(l, fc), h16[:, 500:1000], start=True, stop=True)
                    r16 = tp.tile([P, S], F16, tag="r16")
                    bcol = t_bw1[:, l, fc : fc + 1]
                    if fc % 2 == 0:
                        nc.scalar.activation(_v2(r16), _nv(pf1), AF.Relu, bias=bcol, scale=1.0)
                    else:
                        nc.vector.tensor_scalar(_v2(r16), _nv(pf1), bcol, 0.0, ALU.add, ALU.max)
                    for sc in range(SC):
                        nc.tensor.matmul(pf2[:, sc * 512 : sc * 512 + SCS],
                                         w2(l, fc), r16[:, sc * SCS : (sc + 1) * SCS],
                                         start=(fc == 0), stop=(fc == FC - 1))
                # y2 = h + ff  (bW2 cancels in next norm)
                y2 = xpool.tile([P, S], F32, tag=f"y2_{i}", name=f"y2_{i}")
                nc.vector.scalar_tensor_tensor(_v2(y2), _v2(h16), 0.0, _nv(pf2),
                                               ALU.add, ALU.add,
                                               accum_out=st2[:, 0, j : j + 1])
                y2s[i] = y2
                sq = scr.tile([P, 1024], F32, tag="sqscr")
                nc.scalar.activation(sq[:, 0:S], y2[:], AF.Square, accum_out=st2[:, 1, j : j + 1])
            A2, C2 = _norm_smalls(nc, np_, st2[:, 0], st2[:, 1],
                                  t_g2[:, l : l + 1], t_b2[:, l : l + 1], f"n2_{g0}")
            for i in range(g0, g0 + GRP):
                j = i - g0
                if l < L - 1:
                    nx16 = xpool.tile([D, S], F16, tag=f"x16_{i}{'b' if l % 2 == 0 else ''}",
                                      name=f"nx16_{i}")
                    nc.gpsimd.tensor_scalar(nx16[:], y2s[i][:], A2[:, j : j + 1], C2[:, j : j + 1],
                                            ALU.mult, ALU.add)
                    x16s[i] = nx16
                else:
                    xout = xpool.tile([D, S], F32, tag=f"x32_{i}", name=f"xout_{i}")
                    nc.gpsimd.tensor_scalar(xout[:], y2s[i][:], A2[:, j : j + 1], C2[:, j : j + 1],
                                            ALU.mult, ALU.add)
                    nc.sync.dma_start(g["out32"][i], xout[:])

    ctx.close()


# ------------------------------------------------------------------
# host wrapper
# ------------------------------------------------------------------
_cache = {}


def _get_nc(cs_key):
    if cs_key not in _cache:
        _cache[cs_key] = build_cvrp(list(cs_key))
    return _cache[cs_key]


def prep_inputs(depot_xy, node_xy_demand, dist, log_scale, flag,
                Wd, bd, Wn, bn, Win, bin_, Wout, bout,
                Wq, Wk, Wv, alpha, g1, b1, W1, bW1, W2, bW2, g2, b2):
    import ml_dtypes

    flag = np.asarray(flag)
    cs = tuple(float(np.asarray(log_scale)[0]) * float(a) for a in np.asarray(alpha))
    shared_es = all(abs(c - cs[0]) < 1e-30 for c in cs)

    # es8[b, p, cc, s] = exp(-c_l * dist[b, s, cc*125+p]), fp8-e4m3,
    # cc = consecutive 125-row t-chunks (DoubleRow pairs are (2c, 2c+1)).
    dist_t = np.asarray(dist).transpose(0, 2, 1).reshape(B, TC, TCS, S)
    layers = [cs[0]] if shared_es else list(cs)
    es8 = np.empty((len(layers), B, TCS, TC, S), dtype=ml_dtypes.float8_e4m3)
    for li, c in enumerate(layers):
        es8[li] = np.exp(-c * dist_t).transpose(0, 2, 1, 3).astype(ml_dtypes.float8_e4m3)

    node_t = np.ascontiguousarray(np.asarray(node_xy_demand).transpose(0, 2, 1)).astype(np.float16)
    depot = np.asarray(depot_xy).reshape(B, 2).astype(np.float32)
    flagf = flag.astype(np.float32)

    f16 = lambda a: np.ascontiguousarray(np.asarray(a)).astype(np.float16)
    f32 = lambda a: np.ascontiguousarray(np.asarray(a)).astype(np.float32)
    # per-layer weight blob: [L, D, 1408] = WqT | WkT | WvT | W1T | W2blob
    w2b = np.asarray(W2).transpose(0, 2, 1).reshape(L, FC, P, D).transpose(0, 2, 1, 3).reshape(L, P, FC * D)
    wblob = np.concatenate([
        np.asarray(Wq).transpose(0, 2, 1),
        np.asarray(Wk).transpose(0, 2, 1),
        np.asarray(Wv).transpose(0, 2, 1),
        np.asarray(W1).transpose(0, 2, 1),
        w2b,
    ], axis=2).astype(np.float16)
    sblob = np.concatenate([
        np.stack([np.asarray(bd), np.asarray(bn), np.asarray(bin_), np.asarray(bout)], axis=1),
        np.asarray(bW1).reshape(L, FC, P).transpose(2, 0, 1).reshape(P, L * FC),
        np.asarray(g1).T, np.asarray(b1).T, np.asarray(g2).T, np.asarray(b2).T,
    ], axis=1).astype(np.float32)
    shared = {
        "wblob": np.ascontiguousarray(wblob),
        "wnt": f16(np.asarray(Wn).T),
        "wdt": f32(np.asarray(Wd).T),
        "wint": f32(np.asarray(Win).T),
        "woutt": f32(np.asarray(Wout).T),
        "sblob": np.ascontiguousarray(sblob),
    }
    in_maps = []
    for c in range(NCORES):
        sl = slice(c * IPC, (c + 1) * IPC)
        m = dict(shared)
        m["es8"] = np.ascontiguousarray(es8[:, sl])
        m["node_t"] = node_t[sl]
        m["depot"] = depot[sl]
        m["flagf"] = flagf[sl]
        in_maps.append(m)
    return cs, in_maps


TRACE = False
LAST_RESULT = None


def kernel(**inputs):
    global LAST_RESULT
    cs, in_maps = prep_inputs(**inputs)
    nc = _get_nc(cs)
    res = run_bass_kernel_spmd(nc, in_maps, list(range(NCORES)), trace=TRACE)
    LAST_RESULT = res
    out = np.concatenate([r["out32"] for r in res.results], axis=0)  # [B, D, S]
    return np.ascontiguousarray(out.transpose(0, 2, 1)).astype(np.float32)
